# revision 21
# baseline (speedup 1.0000x reference)
"""Causal attention kernel for TRN2, sharded over batch*heads on 8 NeuronCores.

Problem: B=2, H=16, S=2048, D=64, f32 causal scaled-dot-product attention.

Strategy (per core: 4 heads = 2 head-pairs):
  - Host pre-transposes Q, K to [D, S] (d on partitions), packs two heads
    per 128-partition tile (head A on partitions 0:64, head B on 64:128),
    casts to bf16 (PE runs bf16 at 1 cyc/row vs 4 for f32).
  - QK^T for the two heads runs as two concurrent row-tiled matmuls
    (tile_position auto-derived from base_partition 0 / 64).
  - Host appends a ones-column to V so the softmax denominator falls out of
    the same PE matmul that computes exp(S)@V (M = 65 stationary columns).
  - Work unit: (pair, q-quarter qq of 512, k-tile kt<=4qq+3) strip of
    scoresT [128 k, 2 heads, W<=512 q] in PSUM; one exp ACTIVATE covers both
    heads via a [128, 2, W] access pattern straight out of PSUM (scale=1/8
    folded in); no max-subtraction (scores ~ N(0,1), exp cannot overflow);
    diagonal 128x128 blocks masked by one bf16 triu multiply on VectorE for
    both heads.
  - PSUM (8 banks): scores triple-buffered (3 x 2 banks) so QK always runs
    two groups ahead of exp; one single-buffered [65, 2, 512] out accumulator
    (2 banks) - the AV-side drain wait is absorbed by the ex buffers and never
    blocks ScalarE, which is the bottleneck engine (~74 us of exp).
  - Inputs stream in big chunks over the sync/gpsimd/vector HWDGE queues in
    consumption order (one queue spreads over all 16 DMA engines; the cost
    that matters is the ~0.7us descriptor-gen per dma_start on the issuing
    sequencer). ScalarE's queue carries only the two startup-critical q
    chunks so descriptor-gen never delays ACTIVATEs.
  - Device ships unnormalized [65, S] per head in bf16 (rows 0-63
    numerator^T, row 64 denominator); host divides and transposes back.
    Final quarter's PSUM->SBUF copies are split across ScalarE+VectorE and
    its output DMA over all four queues to shorten the drain tail.
  - Measured: 88.0 us on silicon (HAM pre-warm ladder: 88.0 with 6 warm
    pairs, 88.3 with 7, 88.9 with 8, 91.5 without; baseline 91.0-93.0),
    rel err 4.6e-3 (gate 2e-2). The chip's
    sustained-power downclock adds up to +20% run-to-run drift after many
    back-to-back runs (identical NEFF: 88.8us cold, 106-110us hot; the
    ACTIVATEs themselves slow down, not just the HAM-gated PE) - compare
    variants only back-to-back.
  - Rejected experiments, all hardware-measured: DVE Schraudolph exp2
    offload of the diag strips (DVE_OFFLOAD_DIAG flag; 96.5us - extra
    VectorE activity deepens the chip power throttle and slows the PE);
    gpsimd as a third exp engine (Pool-engine TENSOR_SCALAR fails in the
    bass->NEFF lowering); fp8e4m3 DoubleRow AV / fp8 QK (rel err 2-4e-2,
    over the gate: dominant-key softmax rows expose the 3-6% quantization
    error of V directly); batching activates via 3-bank [128,3,512] score
    tiles (119us - bigger PSUM-src ACTIVATEs run SLOWER per element, and
    two concurrent row-tiled matmuls writing one PSUM bank hang the chip);
    ending the schedule with a small quarter or draining the final quarter
    piecewise (93-95us - the PE is in-order, so the last quarter's AV
    backlog must BE the tail, overlapped with the output drain).
"""

import numpy as np
import ml_dtypes

B, H, S, D = 2, 16, 2048, 64
NCORES = 8
HPC = (B * H) // NCORES  # heads per core = 4
NPAIR = HPC // 2  # head pairs per core = 2
NKT = S // 128  # 16 k-tiles per head
QQ = 512  # q quarter width (one PSUM bank per head)
NQQ = S // QQ
BF16 = ml_dtypes.bfloat16
DVE_OFFLOAD_DIAG = False

_prog = None


def _build_program():
    import concourse.tile as tile
    from concourse import bacc, mybir

    nc = bacc.Bacc(
        "TRN2",
        target_bir_lowering=False,
        debug=False,
        enable_asserts=False,
        num_devices=NCORES,
    )
    # paired layouts: [pair, 128, S] with head 2p on partitions 0:64, head
    # 2p+1 on partitions 64:128
    qT = nc.dram_tensor("qT", [NPAIR, 128, S], mybir.dt.bfloat16, kind="ExternalInput").ap()
    kT = nc.dram_tensor("kT", [NPAIR, 128, S], mybir.dt.bfloat16, kind="ExternalInput").ap()
    vp = nc.dram_tensor("vp", [HPC, 128, NKT, D + 1], mybir.dt.bfloat16, kind="ExternalInput").ap()
    mk = nc.dram_tensor("mk", [128, 128], mybir.dt.bfloat16, kind="ExternalInput").ap()
    o = nc.dram_tensor("o", [HPC, D + 1, S], mybir.dt.bfloat16, kind="ExternalOutput").ap()

    # Schraudolph exp2 constants for the optional DVE offload:
    # bitcast(int16(x*A + B)) as bf16 ~= exp(x/8) with ~3% piecewise-linear
    # error that mostly cancels in the softmax normalization.
    EXP2_A = 128.0 / float(np.log(2.0)) / 8.0
    EXP2_B = 16256.0 - 366393.0 / 65536.0

    with tile.TileContext(nc) as tc:
        with (
            tc.tile_pool(name="inputs", bufs=1) as inputs,
            tc.tile_pool(name="expp", bufs=10) as expp,
            tc.tile_pool(name="scp", bufs=2, space="PSUM") as scp,
            tc.tile_pool(name="outp", bufs=1, space="PSUM") as outp,
            tc.tile_pool(name="outsb", bufs=6) as outsb,
        ):
            mkt = inputs.tile([128, 128], mybir.dt.bfloat16, tag="mask")
            qts, kts_, vts = [], [], []
            for p in range(NPAIR):
                qt = inputs.tile([128, S], mybir.dt.bfloat16, tag=f"q{p}")
                kt = inputs.tile([128, S], mybir.dt.bfloat16, tag=f"k{p}")
                va = inputs.tile([128, NKT, D + 1], mybir.dt.bfloat16, tag=f"va{p}")
                vb = inputs.tile([128, NKT, D + 1], mybir.dt.bfloat16, tag=f"vb{p}")
                qts.append(qt)
                kts_.append(kt)
                vts.append((va, vb))

            # HAM pre-warm: dependency-free dummy matmuls on scratch SBUF
            # (never DMA'd -> no WAR coupling with the critical startup
            # transfers) keep the PE busy through its ~3.4us activity window
            # during the startup DMA wait, so the first real QKs run at
            # 2.4GHz instead of the cold 1.2GHz gate. The warm psum tile is
            # one scp rotation slot; its WAW with a later real group resolves
            # long before that group's data arrives.
            scratch = inputs.tile([128, 512], mybir.dt.bfloat16, tag="scratch")
            nc.vector.memset(scratch, 0)
            warm = scp.tile(
                [128, 2, QQ], mybir.dt.float32, tag="sc", name="warm", bufs=3
            )
            # 6 cold pairs ~= 2.6us: sized so the burst ends at first-QK
            # data arrival; the real QKs then complete the HAM window
            # (8 pairs measured overshooting arrival by ~1.2us)
            for _w in range(6):
                for j in range(2):
                    pb = 64 * j
                    nc.tensor.matmul(
                        warm[:, j, :],
                        scratch[pb : pb + 64, 0:128],
                        scratch[pb : pb + 64, 0:512],
                        start=True,
                        stop=True,
                    )

            # Input DMA schedule, in consumption order of the group schedule
            # below.  sync + gpsimd + vector queues carry the stream (their
            # sequencers are otherwise idle early); scalar only gets the two
            # chunks the very first QK needs, so ACTIVATE dispatch is never
            # stuck behind descriptor generation.
            # wave 1: ONLY what QK/AV group 0 needs. The 16 DMA engines are
            # shared across queues with no priority, so any bulk issued in
            # parallel delays the critical chunks' tail packets (measured
            # +2.7us on the first exp).
            nc.scalar.dma_start(qts[0][:, 0:512], qT[0][:, 0:512])
            nc.sync.dma_start(kts_[0][:, 0:128], kT[0][:, 0:128])
            nc.gpsimd.dma_start(vts[0][0][:, 0:4], vp[0][:, 0:4])
            nc.sync.dma_start(kts_[0][:, 128:512], kT[0][:, 128:512])
            nc.gpsimd.dma_start(vts[0][1][:, 0:4], vp[1][:, 0:4])
            nc.gpsimd.dma_start(mkt[:], mk)
            # (0,1)
            nc.sync.dma_start(qts[0][:, 512:1024], qT[0][:, 512:1024])
            nc.sync.dma_start(kts_[0][:, 512:1024], kT[0][:, 512:1024])
            nc.gpsimd.dma_start(vts[0][0][:, 4:8], vp[0][:, 4:8])
            nc.gpsimd.dma_start(vts[0][1][:, 4:8], vp[1][:, 4:8])
            # (1,0)
            nc.gpsimd.dma_start(qts[1][:, 0:512], qT[1][:, 0:512])
            nc.sync.dma_start(kts_[1][:, 0:512], kT[1][:, 0:512])
            nc.gpsimd.dma_start(vts[1][0][:, 0:4], vp[2][:, 0:4])
            nc.gpsimd.dma_start(vts[1][1][:, 0:4], vp[3][:, 0:4])
            # (0,2)
            nc.sync.dma_start(qts[0][:, 1024:1536], qT[0][:, 1024:1536])
            nc.sync.dma_start(kts_[0][:, 1024:1536], kT[0][:, 1024:1536])
            nc.gpsimd.dma_start(vts[0][0][:, 8:12], vp[0][:, 8:12])
            nc.gpsimd.dma_start(vts[0][1][:, 8:12], vp[1][:, 8:12])
            # (1,1)
            nc.gpsimd.dma_start(qts[1][:, 512:1024], qT[1][:, 512:1024])
            nc.sync.dma_start(kts_[1][:, 512:1024], kT[1][:, 512:1024])
            nc.gpsimd.dma_start(vts[1][0][:, 4:8], vp[2][:, 4:8])
            nc.gpsimd.dma_start(vts[1][1][:, 4:8], vp[3][:, 4:8])
            # (0,3)
            nc.sync.dma_start(qts[0][:, 1536:2048], qT[0][:, 1536:2048])
            nc.sync.dma_start(kts_[0][:, 1536:2048], kT[0][:, 1536:2048])
            nc.gpsimd.dma_start(vts[0][0][:, 12:16], vp[0][:, 12:16])
            nc.gpsimd.dma_start(vts[0][1][:, 12:16], vp[1][:, 12:16])
            # (1,2)
            nc.gpsimd.dma_start(qts[1][:, 1024:1536], qT[1][:, 1024:1536])
            nc.sync.dma_start(kts_[1][:, 1024:1536], kT[1][:, 1024:1536])
            nc.gpsimd.dma_start(vts[1][0][:, 8:12], vp[2][:, 8:12])
            nc.gpsimd.dma_start(vts[1][1][:, 8:12], vp[3][:, 8:12])
            # (1,3)
            nc.gpsimd.dma_start(qts[1][:, 1536:2048], qT[1][:, 1536:2048])
            nc.sync.dma_start(kts_[1][:, 1536:2048], kT[1][:, 1536:2048])
            nc.gpsimd.dma_start(vts[1][0][:, 12:16], vp[2][:, 12:16])
            nc.gpsimd.dma_start(vts[1][1][:, 12:16], vp[3][:, 12:16])

            osbs = {}
            for p in range(NPAIR):
                for jj in range(2):
                    osbs[(p, jj)] = outsb.tile(
                        [D + 1, S], mybir.dt.bfloat16, tag="osb", name=f"osb{p}_{jj}"
                    )
            # interleave the two pairs' quarters so both engines always have
            # independent work to fill dependency gaps
            order = [(0, 0), (0, 1), (1, 0), (0, 2), (1, 1), (0, 3), (1, 2), (1, 3)]

            # flatten (pair, quarter) into a list of strip-group records:
            # (p, qq, group, engine, fused_mask). Diagonal records run their
            # exp on DVE with the causal mask FUSED into the Schraudolph
            # bias (masked columns get B-15000 -> bf16 bits ~2^-110 ~ 0),
            # removing the separate mask TT; they are interleaved among the
            # full-width records so neither exp engine ever runs a burst.
            # A slice of full records also goes to DVE to balance the two
            # engines (Scalar exp ~1.12us/rec vs DVE ~1.3-1.4us/rec + casts).
            # every 5th+3rd record's exp runs on DVE (2-of-5 split measured
            # fastest: Scalar exp ~1.12us/rec vs DVE ~1.3us/rec, and DVE
            # also carries the masks and out-copies). Interleaving diag
            # records mid-quarter + fusing masks into the Schraudolph bias
            # measured SLOWER (85.7 vs 83.5 cold) - keep original order.
            raw = []
            for p, qq in order:
                groups = [[(kti, 0)] for kti in range(4 * qq)]
                groups.append([(4 * qq, 0)])  # W=512 diagonal
                groups.append([(4 * qq + 2, 0)])  # W=256 diagonal
                groups.append([(4 * qq + 1, 0), (4 * qq + 3, 384)])
                for group in groups:
                    raw.append((p, qq, group))
            all_groups = [
                (p, qq, g, "V" if i % 5 in (2, 4) else "S", None)
                for i, (p, qq, g) in enumerate(raw)
            ]

            first_rec, last_rec = {}, {}
            for idx, (p, qq, group, _e, _f) in enumerate(all_groups):
                first_rec.setdefault((p, qq), idx)
                last_rec[(p, qq)] = idx

            def emit_qk(p, qq, group):
                qt, kt = qts[p], kts_[p]
                q0 = QQ * qq
                sc = scp.tile(
                    [128, 2, QQ], mybir.dt.float32, tag="sc", name="sc_tile", bufs=3
                )
                for kti, soff in group:
                    qstart = max(q0, 128 * kti)
                    W = q0 + QQ - qstart
                    for j in range(2):
                        pb = 64 * j
                        nc.tensor.matmul(
                            sc[:, j, soff : soff + W],
                            kt[pb : pb + 64, 128 * kti : 128 * kti + 128],
                            qt[pb : pb + 64, qstart : qstart + W],
                            start=True,
                            stop=True,
                        )
                return sc



            out_ts = {}
            scs = {}
            exs = {}
            N = len(all_groups)
            # Software pipeline: emit QK(r), exp(r-1), AV(r-1-AVLAG) per
            # step so each engine's strict-FIFO queue is already in
            # dependency order — the dual-engine exp otherwise stalls the
            # in-order PE on AVs whose exp hasn't run yet.
            AVLAG = 2

            def emit_exp(rec_i):
                p, qq, group, eng, fused = all_groups[rec_i]
                q0 = QQ * qq
                sc = scs.pop(rec_i)
                wmax = 0
                for kti, soff in group:
                    qstart = max(q0, 128 * kti)
                    W = q0 + QQ - qstart
                    wmax = max(wmax, soff + W)
                ex = expp.tile([128, 2, QQ], mybir.dt.bfloat16, tag="ex")
                # Dual-engine exp: DVE records run a one-op Schraudolph exp2
                # (int16(x*A+B) bits are the bf16 of ~exp(x/8); ~1.5%
                # piecewise-linear error that largely cancels in the softmax
                # ratio). Diagonal records fold the causal mask into the
                # Schraudolph bias tile; Scalar diag records keep a TT mask.
                if eng == "V":
                    nc.vector.tensor_scalar(
                        ex[:, :, :wmax].bitcast(mybir.dt.int16),
                        sc[:, :, :wmax],
                        EXP2_A,
                        EXP2_B,
                        mybir.AluOpType.mult,
                        mybir.AluOpType.add,
                    )
                else:
                    nc.scalar.activation(
                        ex[:, :, :wmax],
                        sc[:, :, :wmax],
                        mybir.ActivationFunctionType.Exp,
                        scale=0.125,
                    )
                for kti, soff in group:
                    if max(q0, 128 * kti) == 128 * kti:
                        # diagonal block of both heads: zero out k > q
                        nc.vector.tensor_mul(
                            ex[:, :, soff : soff + 128],
                            ex[:, :, soff : soff + 128],
                            mkt[:, None, :].to_broadcast((128, 2, 128)),
                        )
                exs[rec_i] = ex

            def emit_av(rec_i):
                p, qq, group, _eng, _fused = all_groups[rec_i]
                q0 = QQ * qq
                ex = exs.pop(rec_i)
                if rec_i == first_rec[(p, qq)]:
                    out_ts[(p, qq)] = outp.tile(
                        [D + 1, 2, QQ],
                        mybir.dt.float32,
                        tag="out",
                        name=f"out{p}_{qq}",
                    )
                out_t = out_ts[(p, qq)]
                for kti, soff in group:
                    qstart = max(q0, 128 * kti)
                    W = q0 + QQ - qstart
                    off = qstart - q0
                    last = rec_i == last_rec[(p, qq)] and (kti, soff) == group[-1]
                    for j in range(2):
                        nc.tensor.matmul(
                            out_t[:, j, off : off + W],
                            vts[p][j][:, kti, :],
                            ex[:, j, soff : soff + W],
                            start=(
                                rec_i == first_rec[(p, qq)]
                                and (kti, soff) == group[0]
                            ),
                            stop=last,
                            skip_group_check=True,
                        )
                if rec_i == last_rec[(p, qq)]:
                    is_final = rec_i == N - 1
                    for j in range(2):
                        osb = osbs[(p, j)]
                        if is_final and j == 1:
                            nc.scalar.copy(osb[:, q0 : q0 + QQ], out_t[:, j, :])
                        else:
                            nc.vector.tensor_copy(osb[:, q0 : q0 + QQ], out_t[:, j, :])
                        if is_final:
                            # drain tail: 4 parallel queue generations
                            hq = QQ // 2
                            engs = (nc.sync, nc.gpsimd) if j == 0 else (nc.scalar, nc.sync)
                            engs[0].dma_start(
                                o[2 * p + j][:, q0 : q0 + hq], osb[:, q0 : q0 + hq]
                            )
                            engs[1].dma_start(
                                o[2 * p + j][:, q0 + hq : q0 + QQ],
                                osb[:, q0 + hq : q0 + QQ],
                            )
                        else:
                            eng = nc.sync if j == 0 else nc.gpsimd
                            eng.dma_start(
                                o[2 * p + j][:, q0 : q0 + QQ], osb[:, q0 : q0 + QQ]
                            )

            for step in range(N + 2 + AVLAG):
                if step < N:
                    p, qq, group, _eng, _fused = all_groups[step]
                    scs[step] = emit_qk(p, qq, group)
                if 0 <= step - 1 < N:
                    emit_exp(step - 1)
                if 0 <= step - 1 - AVLAG < N:
                    emit_av(step - 1 - AVLAG)

    nc.compile()
    return nc


def _get_program():
    global _prog
    if _prog is None:
        _prog = _build_program()
    return _prog


def _prep_in_maps(q, k, v):
    """Build the 8 per-core input maps from full f32 q, k, v."""
    qf = np.ascontiguousarray(q.reshape(B * H, S, D))
    kf = np.ascontiguousarray(k.reshape(B * H, S, D))
    vf = np.ascontiguousarray(v.reshape(B * H, S, D))
    mask = np.triu(np.ones((128, 128), np.float32)).astype(BF16)
    in_maps = []
    for i in range(NCORES):
        sl = slice(HPC * i, HPC * (i + 1))
        # [HPC, D, S] transposed heads, packed pairwise onto 128 partitions
        qT = qf[sl].transpose(0, 2, 1).astype(BF16).reshape(NPAIR, 128, S)
        kT = kf[sl].transpose(0, 2, 1).astype(BF16).reshape(NPAIR, 128, S)
        vpp = np.ones((HPC, 128, NKT, D + 1), dtype=BF16)
        vpp[:, :, :, :D] = (
            vf[sl].reshape(HPC, NKT, 128, D).transpose(0, 2, 1, 3).astype(BF16)
        )
        in_maps.append({"qT": qT, "kT": kT, "vp": vpp, "mk": mask})
    return in_maps


def _postprocess(results):
    """results: list of 8 dicts with 'o' [HPC, D+1, S] bf16 -> full output."""
    o = np.stack([np.asarray(r["o"], dtype=np.float32) for r in results])
    o = o.reshape(B * H, D + 1, S)
    num = o[:, :D, :]  # [BH, D, S]
    den = o[:, D : D + 1, :]  # [BH, 1, S]
    out = (num / den).transpose(0, 2, 1)  # [BH, S, D]
    return np.ascontiguousarray(out.reshape(B, H, S, D).astype(np.float32))


def run(q, k, v, trace=False, **kwargs):
    from concourse.bass_utils import run_bass_kernel_spmd

    nc = _get_program()
    in_maps = _prep_in_maps(q, k, v)
    res = run_bass_kernel_spmd(
        nc, in_maps, core_ids=list(range(NCORES)), trace=trace, **kwargs
    )
    return _postprocess(res.results), res


def kernel(q, k, v):
    out, _ = run(np.asarray(q), np.asarray(k), np.asarray(v))
    return out



# revision 25
# speedup vs baseline: 1.0359x; 1.0359x over previous
"""Causal attention kernel for TRN2, sharded over batch*heads on 8 NeuronCores.

Problem: B=2, H=16, S=2048, D=64, f32 causal scaled-dot-product attention.

Strategy (per core: 4 heads = 2 head-pairs):
  - Host pre-transposes Q, K to [D, S] (d on partitions), packs two heads
    per 128-partition tile (head A on partitions 0:64, head B on 64:128),
    casts to bf16 (PE runs bf16 at 1 cyc/row vs 4 for f32).
  - QK^T for the two heads runs as two concurrent row-tiled matmuls
    (tile_position auto-derived from base_partition 0 / 64).
  - Host appends a ones-column to V so the softmax denominator falls out of
    the same PE matmul that computes exp(S)@V (M = 65 stationary columns).
  - Work unit: (pair, q-quarter qq of 512, k-tile kt<=4qq+3) strip of
    scoresT [128 k, 2 heads, W<=512 q] in PSUM; one exp ACTIVATE covers both
    heads via a [128, 2, W] access pattern straight out of PSUM (scale=1/8
    folded in); no max-subtraction (scores ~ N(0,1), exp cannot overflow);
    diagonal 128x128 blocks masked by one bf16 triu multiply on VectorE for
    both heads.
  - PSUM (8 banks): scores triple-buffered (3 x 2 banks) so QK always runs
    two groups ahead of exp; one single-buffered [65, 2, 512] out accumulator
    (2 banks) - the AV-side drain wait is absorbed by the ex buffers and never
    blocks ScalarE, which is the bottleneck engine (~74 us of exp).
  - Inputs stream in big chunks over the sync/gpsimd/vector HWDGE queues in
    consumption order (one queue spreads over all 16 DMA engines; the cost
    that matters is the ~0.7us descriptor-gen per dma_start on the issuing
    sequencer). ScalarE's queue carries only the two startup-critical q
    chunks so descriptor-gen never delays ACTIVATEs.
  - Device ships unnormalized [65, S] per head in bf16 (rows 0-63
    numerator^T, row 64 denominator); host divides and transposes back.
    Final quarter's PSUM->SBUF copies are split across ScalarE+VectorE and
    its output DMA over all four queues to shorten the drain tail.
  - Measured: 88.0 us on silicon (HAM pre-warm ladder: 88.0 with 6 warm
    pairs, 88.3 with 7, 88.9 with 8, 91.5 without; baseline 91.0-93.0),
    rel err 4.6e-3 (gate 2e-2). The chip's
    sustained-power downclock adds up to +20% run-to-run drift after many
    back-to-back runs (identical NEFF: 88.8us cold, 106-110us hot; the
    ACTIVATEs themselves slow down, not just the HAM-gated PE) - compare
    variants only back-to-back.
  - Rejected experiments, all hardware-measured: DVE Schraudolph exp2
    offload of the diag strips (DVE_OFFLOAD_DIAG flag; 96.5us - extra
    VectorE activity deepens the chip power throttle and slows the PE);
    gpsimd as a third exp engine (Pool-engine TENSOR_SCALAR fails in the
    bass->NEFF lowering); fp8e4m3 DoubleRow AV / fp8 QK (rel err 2-4e-2,
    over the gate: dominant-key softmax rows expose the 3-6% quantization
    error of V directly); batching activates via 3-bank [128,3,512] score
    tiles (119us - bigger PSUM-src ACTIVATEs run SLOWER per element, and
    two concurrent row-tiled matmuls writing one PSUM bank hang the chip);
    ending the schedule with a small quarter or draining the final quarter
    piecewise (93-95us - the PE is in-order, so the last quarter's AV
    backlog must BE the tail, overlapped with the output drain).
"""

import numpy as np
import ml_dtypes

B, H, S, D = 2, 16, 2048, 64
NCORES = 8
HPC = (B * H) // NCORES  # heads per core = 4
NPAIR = HPC // 2  # head pairs per core = 2
NKT = S // 128  # 16 k-tiles per head
QQ = 512  # q quarter width (one PSUM bank per head)
NQQ = S // QQ
BF16 = ml_dtypes.bfloat16
DVE_OFFLOAD_DIAG = False

_prog = None


def _build_program():
    import concourse.tile as tile
    from concourse import bacc, mybir

    nc = bacc.Bacc(
        "TRN2",
        target_bir_lowering=False,
        debug=False,
        enable_asserts=False,
        num_devices=NCORES,
    )
    # paired layouts: [pair, 128, S] with head 2p on partitions 0:64, head
    # 2p+1 on partitions 64:128
    qT = nc.dram_tensor("qT", [NPAIR, 128, S], mybir.dt.bfloat16, kind="ExternalInput").ap()
    kT = nc.dram_tensor("kT", [NPAIR, 128, S], mybir.dt.bfloat16, kind="ExternalInput").ap()
    vp = nc.dram_tensor("vp", [HPC, 128, NKT, D + 1], mybir.dt.bfloat16, kind="ExternalInput").ap()
    mk = nc.dram_tensor("mk", [128, 128], mybir.dt.bfloat16, kind="ExternalInput").ap()
    o = nc.dram_tensor("o", [HPC, D + 1, S], mybir.dt.bfloat16, kind="ExternalOutput").ap()

    # Schraudolph exp2 constants for the optional DVE offload:
    # bitcast(int16(x*A + B)) as bf16 ~= exp(x/8) with ~3% piecewise-linear
    # error that mostly cancels in the softmax normalization.
    EXP2_A = 128.0 / float(np.log(2.0)) / 8.0
    EXP2_B = 16256.0 - 366393.0 / 65536.0

    with tile.TileContext(nc) as tc:
        with (
            tc.tile_pool(name="inputs", bufs=1) as inputs,
            tc.tile_pool(name="expp", bufs=10) as expp,
            tc.tile_pool(name="scp", bufs=2, space="PSUM") as scp,
            tc.tile_pool(name="outp", bufs=1, space="PSUM") as outp,
            tc.tile_pool(name="outsb", bufs=6) as outsb,
        ):
            mkt = inputs.tile([128, 128], mybir.dt.bfloat16, tag="mask")
            qts, kts_, vts = [], [], []
            for p in range(NPAIR):
                qt = inputs.tile([128, S], mybir.dt.bfloat16, tag=f"q{p}")
                kt = inputs.tile([128, S], mybir.dt.bfloat16, tag=f"k{p}")
                va = inputs.tile([128, NKT, D + 1], mybir.dt.bfloat16, tag=f"va{p}")
                vb = inputs.tile([128, NKT, D + 1], mybir.dt.bfloat16, tag=f"vb{p}")
                qts.append(qt)
                kts_.append(kt)
                vts.append((va, vb))

            # HAM pre-warm: dependency-free dummy matmuls on scratch SBUF
            # (never DMA'd -> no WAR coupling with the critical startup
            # transfers) keep the PE busy through its ~3.4us activity window
            # during the startup DMA wait, so the first real QKs run at
            # 2.4GHz instead of the cold 1.2GHz gate. The warm psum tile is
            # one scp rotation slot; its WAW with a later real group resolves
            # long before that group's data arrives.
            # scratch memset runs on gpsimd (earliest-free engine) so the
            # warm ladder can start right after the engine preamble instead
            # of waiting out VectorE's longer preamble.
            scratch = inputs.tile([128, 512], mybir.dt.bfloat16, tag="scratch")
            nc.gpsimd.memset(scratch, 0)
            warm = scp.tile(
                [128, 2, QQ], mybir.dt.float32, tag="sc", name="warm", bufs=3
            )
            # 6 cold pairs ~= 2.6us: sized so the burst ends at first-QK
            # data arrival; the real QKs then complete the HAM window
            # (8 pairs measured overshooting arrival by ~1.2us)
            for _w in range(6):
                for j in range(2):
                    pb = 64 * j
                    nc.tensor.matmul(
                        warm[:, j, :],
                        scratch[pb : pb + 64, 0:128],
                        scratch[pb : pb + 64, 0:512],
                        start=True,
                        stop=True,
                    )

            # Input DMA schedule, in consumption order of the group schedule
            # below.  sync + gpsimd + vector queues carry the stream (their
            # sequencers are otherwise idle early); scalar only gets the two
            # chunks the very first QK needs, so ACTIVATE dispatch is never
            # stuck behind descriptor generation.
            # wave 1: ONLY what QK/AV group 0 needs. The 16 DMA engines are
            # shared across queues with no priority, so any bulk issued in
            # parallel delays the critical chunks' tail packets (measured
            # +2.7us on the first exp).
            # split the startup-critical q chunk across two queues so the
            # two descriptor generations run in parallel
            nc.scalar.dma_start(qts[0][:, 0:256], qT[0][:, 0:256])
            nc.sync.dma_start(kts_[0][:, 0:128], kT[0][:, 0:128])
            nc.sync.dma_start(qts[0][:, 256:512], qT[0][:, 256:512])
            nc.gpsimd.dma_start(vts[0][0][:, 0:4], vp[0][:, 0:4])
            nc.sync.dma_start(kts_[0][:, 128:512], kT[0][:, 128:512])
            nc.gpsimd.dma_start(vts[0][1][:, 0:4], vp[1][:, 0:4])
            nc.gpsimd.dma_start(mkt[:], mk)
            # (0,1)
            nc.sync.dma_start(qts[0][:, 512:1024], qT[0][:, 512:1024])
            nc.sync.dma_start(kts_[0][:, 512:1024], kT[0][:, 512:1024])
            nc.gpsimd.dma_start(vts[0][0][:, 4:8], vp[0][:, 4:8])
            nc.gpsimd.dma_start(vts[0][1][:, 4:8], vp[1][:, 4:8])
            # (1,0)
            nc.gpsimd.dma_start(qts[1][:, 0:512], qT[1][:, 0:512])
            nc.sync.dma_start(kts_[1][:, 0:512], kT[1][:, 0:512])
            nc.gpsimd.dma_start(vts[1][0][:, 0:4], vp[2][:, 0:4])
            nc.gpsimd.dma_start(vts[1][1][:, 0:4], vp[3][:, 0:4])
            # (0,2)
            nc.sync.dma_start(qts[0][:, 1024:1536], qT[0][:, 1024:1536])
            nc.sync.dma_start(kts_[0][:, 1024:1536], kT[0][:, 1024:1536])
            nc.gpsimd.dma_start(vts[0][0][:, 8:12], vp[0][:, 8:12])
            nc.gpsimd.dma_start(vts[0][1][:, 8:12], vp[1][:, 8:12])
            # (1,1)
            nc.gpsimd.dma_start(qts[1][:, 512:1024], qT[1][:, 512:1024])
            nc.sync.dma_start(kts_[1][:, 512:1024], kT[1][:, 512:1024])
            nc.gpsimd.dma_start(vts[1][0][:, 4:8], vp[2][:, 4:8])
            nc.gpsimd.dma_start(vts[1][1][:, 4:8], vp[3][:, 4:8])
            # (0,3)
            nc.sync.dma_start(qts[0][:, 1536:2048], qT[0][:, 1536:2048])
            nc.sync.dma_start(kts_[0][:, 1536:2048], kT[0][:, 1536:2048])
            nc.gpsimd.dma_start(vts[0][0][:, 12:16], vp[0][:, 12:16])
            nc.gpsimd.dma_start(vts[0][1][:, 12:16], vp[1][:, 12:16])
            # (1,2)
            nc.gpsimd.dma_start(qts[1][:, 1024:1536], qT[1][:, 1024:1536])
            nc.sync.dma_start(kts_[1][:, 1024:1536], kT[1][:, 1024:1536])
            nc.gpsimd.dma_start(vts[1][0][:, 8:12], vp[2][:, 8:12])
            nc.gpsimd.dma_start(vts[1][1][:, 8:12], vp[3][:, 8:12])
            # (1,3)
            nc.gpsimd.dma_start(qts[1][:, 1536:2048], qT[1][:, 1536:2048])
            nc.sync.dma_start(kts_[1][:, 1536:2048], kT[1][:, 1536:2048])
            nc.gpsimd.dma_start(vts[1][0][:, 12:16], vp[2][:, 12:16])
            nc.gpsimd.dma_start(vts[1][1][:, 12:16], vp[3][:, 12:16])

            osbs = {}
            for p in range(NPAIR):
                for jj in range(2):
                    osbs[(p, jj)] = outsb.tile(
                        [D + 1, S], mybir.dt.bfloat16, tag="osb", name=f"osb{p}_{jj}"
                    )
            # interleave the two pairs' quarters so both engines always have
            # independent work to fill dependency gaps
            order = [(0, 0), (0, 1), (1, 0), (0, 2), (1, 1), (0, 3), (1, 2), (1, 3)]

            # flatten (pair, quarter) into a list of strip-group records:
            # (p, qq, group, engine, fused_mask). Diagonal records run their
            # exp on DVE with the causal mask FUSED into the Schraudolph
            # bias (masked columns get B-15000 -> bf16 bits ~2^-110 ~ 0),
            # removing the separate mask TT; they are interleaved among the
            # full-width records so neither exp engine ever runs a burst.
            # A slice of full records also goes to DVE to balance the two
            # engines (Scalar exp ~1.12us/rec vs DVE ~1.3-1.4us/rec + casts).
            # every 5th+3rd record's exp runs on DVE (2-of-5 split measured
            # fastest: Scalar exp ~1.12us/rec vs DVE ~1.3us/rec, and DVE
            # also carries the masks and out-copies). Interleaving diag
            # records mid-quarter + fusing masks into the Schraudolph bias
            # measured SLOWER (85.7 vs 83.5 cold) - keep original order.
            raw = []
            for p, qq in order:
                groups = [[(kti, 0)] for kti in range(4 * qq)]
                groups.append([(4 * qq, 0)])  # W=512 diagonal
                groups.append([(4 * qq + 2, 0)])  # W=256 diagonal
                groups.append([(4 * qq + 1, 0), (4 * qq + 3, 384)])
                for group in groups:
                    raw.append((p, qq, group))
            all_groups = [
                (p, qq, g, "V" if i % 5 in (2, 4) else "S", None)
                for i, (p, qq, g) in enumerate(raw)
            ]

            first_rec, last_rec = {}, {}
            for idx, (p, qq, group, _e, _f) in enumerate(all_groups):
                first_rec.setdefault((p, qq), idx)
                last_rec[(p, qq)] = idx

            def emit_qk(p, qq, group):
                qt, kt = qts[p], kts_[p]
                q0 = QQ * qq
                sc = scp.tile(
                    [128, 2, QQ], mybir.dt.float32, tag="sc", name="sc_tile", bufs=3
                )
                for kti, soff in group:
                    qstart = max(q0, 128 * kti)
                    W = q0 + QQ - qstart
                    for j in range(2):
                        pb = 64 * j
                        nc.tensor.matmul(
                            sc[:, j, soff : soff + W],
                            kt[pb : pb + 64, 128 * kti : 128 * kti + 128],
                            qt[pb : pb + 64, qstart : qstart + W],
                            start=True,
                            stop=True,
                        )
                return sc



            out_ts = {}
            scs = {}
            exs = {}
            N = len(all_groups)
            # Software pipeline: emit QK(r), exp(r-1), AV(r-1-AVLAG) per
            # step so each engine's strict-FIFO queue is already in
            # dependency order — the dual-engine exp otherwise stalls the
            # in-order PE on AVs whose exp hasn't run yet.
            AVLAG = 2

            def emit_exp(rec_i):
                p, qq, group, eng, fused = all_groups[rec_i]
                q0 = QQ * qq
                sc = scs.pop(rec_i)
                wmax = 0
                for kti, soff in group:
                    qstart = max(q0, 128 * kti)
                    W = q0 + QQ - qstart
                    wmax = max(wmax, soff + W)
                ex = expp.tile([128, 2, QQ], mybir.dt.bfloat16, tag="ex")
                # Dual-engine exp: DVE records run a one-op Schraudolph exp2
                # (int16(x*A+B) bits are the bf16 of ~exp(x/8); ~1.5%
                # piecewise-linear error that largely cancels in the softmax
                # ratio). Diagonal records fold the causal mask into the
                # Schraudolph bias tile; Scalar diag records keep a TT mask.
                if eng == "V":
                    nc.vector.tensor_scalar(
                        ex[:, :, :wmax].bitcast(mybir.dt.int16),
                        sc[:, :, :wmax],
                        EXP2_A,
                        EXP2_B,
                        mybir.AluOpType.mult,
                        mybir.AluOpType.add,
                    )
                else:
                    nc.scalar.activation(
                        ex[:, :, :wmax],
                        sc[:, :, :wmax],
                        mybir.ActivationFunctionType.Exp,
                        scale=0.125,
                    )
                for kti, soff in group:
                    if max(q0, 128 * kti) == 128 * kti:
                        # diagonal block of both heads: zero out k > q
                        nc.vector.tensor_mul(
                            ex[:, :, soff : soff + 128],
                            ex[:, :, soff : soff + 128],
                            mkt[:, None, :].to_broadcast((128, 2, 128)),
                        )
                exs[rec_i] = ex

            def emit_av(rec_i):
                p, qq, group, _eng, _fused = all_groups[rec_i]
                q0 = QQ * qq
                ex = exs.pop(rec_i)
                if rec_i == first_rec[(p, qq)]:
                    out_ts[(p, qq)] = outp.tile(
                        [D + 1, 2, QQ],
                        mybir.dt.float32,
                        tag="out",
                        name=f"out{p}_{qq}",
                    )
                out_t = out_ts[(p, qq)]
                for kti, soff in group:
                    qstart = max(q0, 128 * kti)
                    W = q0 + QQ - qstart
                    off = qstart - q0
                    last = rec_i == last_rec[(p, qq)] and (kti, soff) == group[-1]
                    for j in range(2):
                        nc.tensor.matmul(
                            out_t[:, j, off : off + W],
                            vts[p][j][:, kti, :],
                            ex[:, j, soff : soff + W],
                            start=(
                                rec_i == first_rec[(p, qq)]
                                and (kti, soff) == group[0]
                            ),
                            stop=last,
                            skip_group_check=True,
                        )
                if rec_i == last_rec[(p, qq)]:
                    is_final = rec_i == N - 1
                    for j in range(2):
                        osb = osbs[(p, j)]
                        if is_final and j == 1:
                            nc.scalar.copy(osb[:, q0 : q0 + QQ], out_t[:, j, :])
                        else:
                            nc.vector.tensor_copy(osb[:, q0 : q0 + QQ], out_t[:, j, :])
                        if is_final:
                            # drain tail: 4 parallel queue generations
                            hq = QQ // 2
                            engs = (nc.sync, nc.gpsimd) if j == 0 else (nc.scalar, nc.sync)
                            engs[0].dma_start(
                                o[2 * p + j][:, q0 : q0 + hq], osb[:, q0 : q0 + hq]
                            )
                            engs[1].dma_start(
                                o[2 * p + j][:, q0 + hq : q0 + QQ],
                                osb[:, q0 + hq : q0 + QQ],
                            )
                        else:
                            eng = nc.sync if j == 0 else nc.gpsimd
                            eng.dma_start(
                                o[2 * p + j][:, q0 : q0 + QQ], osb[:, q0 : q0 + QQ]
                            )

            for step in range(N + 2 + AVLAG):
                if step < N:
                    p, qq, group, _eng, _fused = all_groups[step]
                    scs[step] = emit_qk(p, qq, group)
                if 0 <= step - 1 < N:
                    emit_exp(step - 1)
                if 0 <= step - 1 - AVLAG < N:
                    emit_av(step - 1 - AVLAG)

    nc.compile()
    return nc


def _get_program():
    global _prog
    if _prog is None:
        _prog = _build_program()
    return _prog


def _prep_in_maps(q, k, v):
    """Build the 8 per-core input maps from full f32 q, k, v."""
    qf = np.ascontiguousarray(q.reshape(B * H, S, D))
    kf = np.ascontiguousarray(k.reshape(B * H, S, D))
    vf = np.ascontiguousarray(v.reshape(B * H, S, D))
    mask = np.triu(np.ones((128, 128), np.float32)).astype(BF16)
    in_maps = []
    for i in range(NCORES):
        sl = slice(HPC * i, HPC * (i + 1))
        # [HPC, D, S] transposed heads, packed pairwise onto 128 partitions
        qT = qf[sl].transpose(0, 2, 1).astype(BF16).reshape(NPAIR, 128, S)
        kT = kf[sl].transpose(0, 2, 1).astype(BF16).reshape(NPAIR, 128, S)
        vpp = np.ones((HPC, 128, NKT, D + 1), dtype=BF16)
        vpp[:, :, :, :D] = (
            vf[sl].reshape(HPC, NKT, 128, D).transpose(0, 2, 1, 3).astype(BF16)
        )
        in_maps.append({"qT": qT, "kT": kT, "vp": vpp, "mk": mask})
    return in_maps


def _postprocess(results):
    """results: list of 8 dicts with 'o' [HPC, D+1, S] bf16 -> full output."""
    o = np.stack([np.asarray(r["o"], dtype=np.float32) for r in results])
    o = o.reshape(B * H, D + 1, S)
    num = o[:, :D, :]  # [BH, D, S]
    den = o[:, D : D + 1, :]  # [BH, 1, S]
    out = (num / den).transpose(0, 2, 1)  # [BH, S, D]
    return np.ascontiguousarray(out.reshape(B, H, S, D).astype(np.float32))


def run(q, k, v, trace=False, **kwargs):
    from concourse.bass_utils import run_bass_kernel_spmd

    nc = _get_program()
    in_maps = _prep_in_maps(q, k, v)
    res = run_bass_kernel_spmd(
        nc, in_maps, core_ids=list(range(NCORES)), trace=trace, **kwargs
    )
    return _postprocess(res.results), res


def kernel(q, k, v):
    out, _ = run(np.asarray(q), np.asarray(k), np.asarray(v))
    return out

